# revision 23
# baseline (speedup 1.0000x reference)
"""Two-way cross-attention block (SuperGlue-style) on 8 trn2 NeuronCores.

Sharding: attention is sharded by head (8 heads -> 8 cores); the MLP /
conv1x1(Wm,W1,W2) + instance-norm part is sharded by sequence position
(2048 -> 8 chunks of 256).  Both batch items are pipelined independently
so that every collective (AllToAll of attention outputs, AllReduce of
instance-norm stats, merged k/v AllToAll for block 2) overlaps the other
batch's compute.

Numerics: matmul inputs bf16 (PE 1 cycle/row), except q/k which are
fp8e4m3 and use the DoubleRow perf mode (0.5 cycles/row) with the head
depth split into two K-tiles of 32; fp32 PSUM accumulation everywhere;
InstanceNorm stats and the residual path stay fp32.  Attention PV is
computed transposed (p stationary, vT moving) so the contraction runs at
full K=128 and softmax normalization becomes a per-partition
tensor_scalar; a PE transpose puts the output back in [d, n] layout with
the v-bias fused into the PSUM->SBUF copy.
Exact-math rewrites vs the reference:
  * v-projection bias applied after softmax normalization (rows sum to 1).
  * W1's conv bias cancels inside the affine-free InstanceNorm -> dropped.
  * softmax without max-subtraction (scores are small, safe in fp32).
"""

import sys

if "/opt/trn_rl_repo" not in sys.path:
    sys.path.insert(0, "/opt/trn_rl_repo")

import ml_dtypes
import numpy as np

import concourse.bass as bass
import concourse.masks as masks
import concourse.mybir as mybir
import concourse.tile as tile
from concourse import bacc
from concourse.bass_utils import run_bass_kernel_spmd

F32 = mybir.dt.float32
BF16 = mybir.dt.bfloat16
FP8 = mybir.dt.float8e4
AF = mybir.ActivationFunctionType
ALU = mybir.AluOpType
AX = mybir.AxisListType
DR = mybir.MatmulPerfMode.DoubleRow
USE_DR = False  # fp8 DoubleRow scores (0.5 cyc/row) vs plain fp8 (1 cyc/row)

B = 2        # batch
F = 512      # feature dim
H = 8        # heads
D = 64       # head depth
N = 2048     # sequence length
NL = N // 8  # per-core position chunk (256)
NCORES = 8
EPS = 1e-5
RG = [list(range(NCORES))]

_CACHE = {}


def build_nc(reps=1):
    nc = bacc.Bacc("TRN2", target_bir_lowering=False, num_devices=NCORES)

    # ---------------- external I/O (per core) ----------------
    src = nc.dram_tensor("src", [B, F, N], BF16, kind="ExternalInput")
    tgt = nc.dram_tensor("tgt", [B, F, N], BF16, kind="ExternalInput")
    x1c = nc.dram_tensor("x1c", [B, F, NL], F32, kind="ExternalInput")
    x2c = nc.dram_tensor("x2c", [B, F, NL], F32, kind="ExternalInput")
    x1b = nc.dram_tensor("x1b", [B, F, NL], BF16, kind="ExternalInput")
    x2b = nc.dram_tensor("x2b", [B, F, NL], BF16, kind="ExternalInput")
    wq = nc.dram_tensor("wq", [F, D], BF16, kind="ExternalInput")
    wk = nc.dram_tensor("wk", [F, D], BF16, kind="ExternalInput")
    wv = nc.dram_tensor("wv", [F, D], BF16, kind="ExternalInput")
    bqk = nc.dram_tensor("bqk", [D, 2], F32, kind="ExternalInput")
    bv = nc.dram_tensor("bv", [D, 1], F32, kind="ExternalInput")
    wmT = nc.dram_tensor("wmT", [F, F], BF16, kind="ExternalInput")
    bm = nc.dram_tensor("bm", [F, 1], F32, kind="ExternalInput")
    w1T = nc.dram_tensor("w1T", [2 * F, 2 * F], BF16, kind="ExternalInput")
    w2T = nc.dram_tensor("w2T", [2 * F, F], BF16, kind="ExternalInput")
    b2 = nc.dram_tensor("b2", [F, 1], F32, kind="ExternalInput")
    # block-2 sender-side projection weights (output channels head-major)
    wk2T = nc.dram_tensor("wk2T", [F, F], BF16, kind="ExternalInput")
    wv2T = nc.dram_tensor("wv2T", [F, F], BF16, kind="ExternalInput")
    bk2 = nc.dram_tensor("bk2", [F, 1], F32, kind="ExternalInput")

    src_out_c = nc.dram_tensor("src_out_c", [B, F, NL], F32, kind="ExternalOutput")
    tgt_out_c = nc.dram_tensor("tgt_out_c", [B, F, NL], F32, kind="ExternalOutput")

    # ---------------- internal DRAM (collectives, per block x batch) ----------
    cc_a_in = [[nc.dram_tensor(f"cc_a_in{i}_{b}", [NCORES, D, NL], BF16)
                for b in range(B)] for i in range(2)]
    a2a_a = [[nc.dram_tensor(f"a2a_a{i}_{b}", [NCORES, D, NL], BF16)
              for b in range(B)] for i in range(2)]
    cc_st_in = [[nc.dram_tensor(f"cc_st_in{i}_{b}", [128, 16], F32)
                 for b in range(B)] for i in range(2)]
    ar_st = [[nc.dram_tensor(f"ar_st{i}_{b}", [128, 16], F32, addr_space="Shared")
              for b in range(B)] for i in range(2)]
    # merged k/v AllToAll for block 2: slot h = [k fp8 | vT bf16 as 2 fp8 planes]
    cc_kv_in = [nc.dram_tensor(f"cc_kv_in{b}", [NCORES, 3, D, NL], FP8)
                for b in range(B)]
    a2a_kv = [nc.dram_tensor(f"a2a_kv{b}", [NCORES, 3, D, NL], FP8)
              for b in range(B)]

    with tile.TileContext(nc) as tc, bass.ExitStack() as ctx:
        # ---------- persistent tiles ----------
        wp = ctx.enter_context(tc.tile_pool(name="weights", bufs=1))
        wq_sb = wp.tile([128, 4, D], BF16, tag="wq")
        wk_sb = wp.tile([128, 4, D], BF16, tag="wk")
        wv_sb = wp.tile([128, 4, D], BF16, tag="wv")
        bqk_sb = wp.tile([D, 2], F32, tag="bqk")
        bv_sb = wp.tile([D, 1], F32, tag="bv")
        wm_sb = wp.tile([128, 4, F], BF16, tag="wm")
        bm_sb = wp.tile([128, 4], F32, tag="bm")
        w1_sb = wp.tile([128, 8, 2 * F], BF16, tag="w1")
        w2_sb = wp.tile([128, 8, F], BF16, tag="w2")
        b2_sb = wp.tile([128, 4], F32, tag="b2")
        wk2_sb = wp.tile([128, 4, F], BF16, tag="wk2")
        wv2_sb = wp.tile([128, 4, F], BF16, tag="wv2")
        bk2_sb = wp.tile([128, 4], F32, tag="bk2")
        ident = wp.tile([128, 128], BF16, tag="ident")
        masks.make_identity(nc, ident[:])

        for t, d_ in ((wq_sb, wq), (wk_sb, wk), (wv_sb, wv)):
            nc.sync.dma_start(out=t[:], in_=d_[:].rearrange("(t p) d -> p t d", p=128))
        nc.sync.dma_start(out=bqk_sb[:], in_=bqk[:])
        nc.sync.dma_start(out=bv_sb[:], in_=bv[:])
        for t, d_ in ((wm_sb, wmT), (w1_sb, w1T), (w2_sb, w2T),
                      (wk2_sb, wk2T), (wv2_sb, wv2T)):
            nc.sync.dma_start(out=t[:], in_=d_[:].rearrange("(t p) o -> p t o", p=128))
        for t, d_ in ((bm_sb, bm), (b2_sb, b2), (bk2_sb, bk2)):
            nc.sync.dma_start(out=t[:],
                              in_=d_[:].rearrange("(t p) one -> p (t one)", p=128))

        def stage_half(dst8, src8, b):
            """fp8 [64,B,N] -> DoubleRow layout [32, 2, B, N] via 2 SBUF DMAs."""
            if not USE_DR:
                return
            nc.sync.dma_start(out=dst8[:, 0, b, :], in_=src8[0:32, b, :])
            nc.sync.dma_start(out=dst8[:, 1, b, :], in_=src8[32:64, b, :])

        def emit_q_proj(qsrc_dram, q_sb, q8, b, name):
            """q8[dlo, dhi, b, n] = fp8(Wq_h @ qsrc + bq_h), streamed."""
            with tc.tile_pool(name=f"tb{name}", bufs=5) as tbp, \
                 tc.tile_pool(name=f"psQ{name}", bufs=4, space="PSUM") as psQ:
                qtiles = []
                for kf in range(4):
                    t = tbp.tile([128, N], BF16, tag="kv")
                    nc.sync.dma_start(out=t[:],
                                      in_=qsrc_dram[b, kf * 128:(kf + 1) * 128, :])
                    qtiles.append(t)
                for nt in range(4):
                    ps = psQ.tile([D, 512], F32, tag="qps")
                    for kf in range(4):
                        nc.tensor.matmul(ps[:], wq_sb[:, kf, :],
                                         qtiles[kf][:, nt * 512:(nt + 1) * 512],
                                         start=(kf == 0), stop=(kf == 3))
                    nc.vector.tensor_scalar(
                        q_sb[:, b, nt * 512:(nt + 1) * 512],
                        ps[:], bqk_sb[:, 0:1], None, ALU.add)
                stage_half(q8, q_sb, b)

        def emit_kv_proj_block0(k_sb, k8, vt_sb, b):
            """Block-0 k (fp8) and vT (bf16) projections from tgt."""
            with tc.tile_pool(name=f"tbKV0_{b}", bufs=5) as tbp, \
                 tc.tile_pool(name=f"psK0_{b}", bufs=4, space="PSUM") as psA, \
                 tc.tile_pool(name=f"psV0_{b}", bufs=4, space="PSUM") as psVT:
                tiles = []
                for kf in range(4):
                    t = tbp.tile([128, N], BF16, tag="kv")
                    nc.sync.dma_start(out=t[:],
                                      in_=tgt[b, kf * 128:(kf + 1) * 128, :])
                    tiles.append(t)
                for nt in range(4):
                    ps = psA.tile([D, 512], F32, tag="kps")
                    for kf in range(4):
                        nc.tensor.matmul(ps[:], wk_sb[:, kf, :],
                                         tiles[kf][:, nt * 512:(nt + 1) * 512],
                                         start=(kf == 0), stop=(kf == 3))
                    nc.vector.tensor_scalar(
                        k_sb[:, b, nt * 512:(nt + 1) * 512],
                        ps[:], bqk_sb[:, 1:2], None, ALU.add)
                for mi in range(16):
                    ps = psVT.tile([128, D], F32, tag="vtps")
                    for kf in range(4):
                        nc.tensor.matmul(ps[:],
                                         tiles[kf][:, mi * 128:(mi + 1) * 128],
                                         wv_sb[:, kf, :],
                                         start=(kf == 0), stop=(kf == 3))
                    nc.vector.tensor_copy(vt_sb[:, b, mi, 0:D], ps[:])
                stage_half(k8, k_sb, b)

        def emit_attention(block, b, qpair, kpair, vt_sb, a_bf, pp, dnp, psS,
                           psPV, psT):
            """scores (fp8 DoubleRow) -> exp -> transposed PV -> [d,n] + bv."""
            q_sb, q8 = qpair
            k_sb, k8 = kpair
            for nb in range(4):
                nsl = slice(nb * 512, (nb + 1) * 512)
                p_sb = pp.tile([128, 16, 512], BF16, tag="p")
                for g in range(8):
                    ps = psS.tile([128, 2, 512], F32, tag="sps")
                    for j in range(2):
                        mi = 2 * g + j
                        if USE_DR:
                            nc.tensor.matmul(
                                ps[:, j, :],
                                k8[:, :, b, mi * 128:(mi + 1) * 128],
                                q8[:, :, b, nsl], start=True, stop=True,
                                perf_mode=DR)
                        else:
                            nc.tensor.matmul(
                                ps[:, j, :],
                                k_sb[:, b, mi * 128:(mi + 1) * 128],
                                q_sb[:, b, nsl], start=True, stop=True)
                    nc.scalar.activation(p_sb[:, 2 * g:2 * g + 2, :], ps[:],
                                         AF.Exp, scale=float(1.0 / np.sqrt(D)))
                for s4 in range(4):
                    pv = psPV.tile([128, D + 1], F32, tag="pvps")
                    for mi in range(16):
                        nc.tensor.matmul(
                            pv[:], p_sb[:, mi, s4 * 128:(s4 + 1) * 128],
                            vt_sb[:, b, mi, :],
                            start=(mi == 0), stop=(mi == 15))
                    rden = dnp.tile([128, 1], F32, tag="rden")
                    nc.vector.reciprocal(rden[:], pv[:, D:D + 1])
                    at_sb = dnp.tile([128, D], BF16, tag="at")
                    nc.vector.tensor_scalar(at_sb[:], pv[:, 0:D], rden[:],
                                            None, ALU.mult)
                    tp = psT.tile([D, 128], BF16, tag="tps")
                    nc.tensor.transpose(tp[:], at_sb[:], ident[:])
                    n0 = nb * 512 + s4 * 128
                    nc.vector.tensor_scalar(a_bf[:, b, n0:n0 + 128], tp[:],
                                            bv_sb[:], None, ALU.add)
            nc.sync.dma_start(
                out=cc_a_in[block][b][:].rearrange("c d n -> d c n"),
                in_=a_bf[:, b, :].rearrange("d (c n) -> d c n", c=NCORES))
            nc.gpsimd.collective_compute(
                "AllToAll", ALU.bypass, replica_groups=RG,
                ins=[cc_a_in[block][b][:]], outs=[a2a_a[block][b][:]])

        def emit_mlp_front(block, b, x_chunk_dram, xb_dram, pools):
            """Wm + W1 + stats, then the stats AllReduce for batch b."""
            (xcp, acp, amp, hp, stp, scp, psC) = pools
            x_sb = xcp.tile([128, 4, NL], F32, tag=f"x{b}")
            xb_sb = xcp.tile([128, 4, NL], BF16, tag=f"xb{b}")
            nc.sync.dma_start(
                out=x_sb[:], in_=x_chunk_dram[b].rearrange("(t p) n -> p t n", p=128))
            nc.sync.dma_start(
                out=xb_sb[:], in_=xb_dram[b].rearrange("(t p) n -> p t n", p=128))
            am_sb = amp.tile([128, 4, NL], BF16, tag=f"am{b}")
            h1_sb = hp.tile([128, 8, NL], BF16, tag=f"h1{b}")
            stats = stp.tile([128, 16], F32, tag=f"st{b}")
            atiles = []
            for g in range(4):
                at = acp.tile([128, NL], BF16, tag=f"ach{b}")
                nc.sync.dma_start(
                    out=at[:],
                    in_=a2a_a[block][b][2 * g:2 * g + 2, :, :].rearrange(
                        "c d n -> (c d) n"))
                atiles.append(at)
            for o in range(4):
                ps = psC.tile([128, NL], F32, tag="cps")
                for g in range(4):
                    nc.tensor.matmul(ps[:], wm_sb[:, g, o * 128:(o + 1) * 128],
                                     atiles[g][:],
                                     start=(g == 0), stop=(g == 3))
                nc.vector.tensor_scalar(am_sb[:, o, :], ps[:],
                                        bm_sb[:, o:o + 1], None, ALU.add)
            for o in range(8):
                ps = psC.tile([128, NL], F32, tag="cps")
                for g in range(8):
                    rhs = (xb_sb[:, g, :] if g < 4 else am_sb[:, g - 4, :])
                    nc.tensor.matmul(ps[:], w1_sb[:, g, o * 128:(o + 1) * 128],
                                     rhs, start=(g == 0), stop=(g == 7))
                nc.vector.tensor_copy(h1_sb[:, o, :], ps[:])
                nc.vector.tensor_reduce(stats[:, 2 * o:2 * o + 1],
                                        h1_sb[:, o, :], AX.X, ALU.add)
                # tensor_tensor_reduce crashes this runtime; ACT Square with
                # accum_out computes the sum of squares.
                scr = scp.tile([128, NL], BF16, tag="sq")
                nc.scalar.activation(
                    scr[:], h1_sb[:, o, :], AF.Square,
                    accum_out=stats[:, 2 * o + 1:2 * o + 2])
            nc.sync.dma_start(out=cc_st_in[block][b][:], in_=stats[:])
            nc.gpsimd.collective_compute(
                "AllReduce", ALU.add, replica_groups=RG,
                ins=[cc_st_in[block][b][:]], outs=[ar_st[block][b][:]])
            return x_sb, xb_sb, h1_sb

        def emit_mlp_back(block, b, x_sb, h1_sb, out_dram, send_kv, pools):
            """norm + relu + W2 + residual + (sender-side kv2 + AllToAll)."""
            (xcp, acp, amp, hp, stp, scp, psC) = pools
            stg = stp.tile([128, 16], F32, tag=f"stg{b}")
            nc.sync.dma_start(out=stg[:], in_=ar_st[block][b][:])
            mean = stp.tile([128, 8], F32, tag=f"mean{b}")
            vtmp = stp.tile([128, 8], F32, tag=f"vtmp{b}")
            rstd = stp.tile([128, 8], F32, tag=f"rstd{b}")
            nbias = stp.tile([128, 8], F32, tag=f"nbias{b}")
            s1v = stg[:].rearrange("p (c two) -> p c two", two=2)
            nc.vector.tensor_scalar(mean[:], s1v[:, :, 0], 1.0 / N, None,
                                    ALU.mult)
            nc.vector.tensor_scalar(vtmp[:], s1v[:, :, 1], 1.0 / N, None,
                                    ALU.mult)
            nc.vector.tensor_mul(nbias[:], mean[:], mean[:])
            nc.vector.tensor_sub(vtmp[:], vtmp[:], nbias[:])
            nc.vector.tensor_scalar(vtmp[:], vtmp[:], EPS, None, ALU.add)
            nc.scalar.sqrt(vtmp[:], vtmp[:])
            nc.vector.reciprocal(rstd[:], vtmp[:])
            nc.vector.tensor_mul(nbias[:], mean[:], rstd[:])
            nc.vector.tensor_scalar(nbias[:], nbias[:], -1.0, None, ALU.mult)
            # IN + relu applied in place (h1 is dead afterwards)
            hr_sb = h1_sb
            for o in range(8):
                nc.scalar.activation(hr_sb[:, o, :], h1_sb[:, o, :],
                                     AF.Relu, bias=nbias[:, o:o + 1],
                                     scale=rstd[:, o:o + 1])
            so_sb = xcp.tile([128, 4, NL], F32, tag=f"so{b}")
            for o in range(4):
                ps = psC.tile([128, NL], F32, tag="cps")
                for g in range(8):
                    nc.tensor.matmul(ps[:], w2_sb[:, g, o * 128:(o + 1) * 128],
                                     hr_sb[:, g, :],
                                     start=(g == 0), stop=(g == 7))
                nc.vector.tensor_scalar(so_sb[:, o, :], ps[:],
                                        b2_sb[:, o:o + 1], None, ALU.add)
                nc.vector.tensor_add(so_sb[:, o, :], so_sb[:, o, :],
                                     x_sb[:, o, :])
                nc.sync.dma_start(
                    out=out_dram[b, o * 128:(o + 1) * 128, :],
                    in_=so_sb[:, o, :])
            if send_kv:
                # sender-side block-2 k (fp8) / vT (bf16) projections from the
                # resident src_out chunk, then one merged AllToAll.
                so_bf = xcp.tile([128, 4, NL], BF16, tag=f"sobf{b}")
                for o in range(4):
                    nc.vector.tensor_copy(so_bf[:, o, :], so_sb[:, o, :])
                k2_sb = xcp.tile([128, 4, NL], FP8, tag=f"k2{b}")
                v2_sb = xcp.tile([128, 2, F], BF16, tag=f"v2t{b}")
                with tc.tile_pool(name=f"psS2_{block}_{b}", bufs=2,
                                  space="PSUM") as psS2:
                    for t in range(4):
                        ps = psS2.tile([128, NL], F32, tag="s2ps")
                        for g in range(4):
                            nc.tensor.matmul(
                                ps[:], wk2_sb[:, g, t * 128:(t + 1) * 128],
                                so_bf[:, g, :],
                                start=(g == 0), stop=(g == 3))
                        nc.vector.tensor_scalar(k2_sb[:, t, :], ps[:],
                                                bk2_sb[:, t:t + 1], None,
                                                ALU.add)
                    for mt in range(2):
                        ps = psS2.tile([128, F], F32, tag="s2ps")
                        for g in range(4):
                            nc.tensor.matmul(
                                ps[:],
                                so_bf[:, g, mt * 128:(mt + 1) * 128],
                                wv2_sb[:, g, :],
                                start=(g == 0), stop=(g == 3))
                        nc.vector.tensor_copy(v2_sb[:, mt, :], ps[:])
                for h in range(NCORES):
                    t, r = h // 2, (h % 2) * D
                    nc.sync.dma_start(out=cc_kv_in[b][h, 0],
                                      in_=k2_sb[r:r + D, t, :])
                    # v bf16 bytes shipped as 2 fp8 planes; layout is the flat
                    # (mt p d2) byte order the receiver unpacks below.
                    nc.sync.dma_start(
                        out=cc_kv_in[b][h, 1:3].rearrange(
                            "two d n -> (two d n)").rearrange(
                            "(mt p d2) -> p mt d2", mt=2, p=128),
                        in_=v2_sb[:, :, h * D:(h + 1) * D].bitcast(FP8))
                nc.gpsimd.collective_compute(
                    "AllToAll", ALU.bypass, replica_groups=RG,
                    ins=[cc_kv_in[b][:]], outs=[a2a_kv[b][:]])

        def emit_kv_assembly_block1(k_sb, k8, vt_sb, b):
            nc.sync.dma_start(
                out=k_sb[:, b, :].rearrange("d (c n) -> d c n", c=NCORES),
                in_=a2a_kv[b][:, 0, :, :].rearrange("c d n -> d c n"))
            stage_half(k8, k_sb, b)
            for c in range(NCORES):
                nc.sync.dma_start(
                    out=vt_sb[:, b, 2 * c:2 * c + 2, 0:D].bitcast(FP8),
                    in_=a2a_kv[b][c, 1:3].rearrange(
                        "two d n -> (two d n)").rearrange(
                        "(mt p d2) -> p mt d2", mt=2, p=128))

        def mlp_pools(block, stack):
            cms = (tc.tile_pool(name=f"xC{block}", bufs=1),
                   tc.tile_pool(name=f"aC{block}", bufs=8),
                   tc.tile_pool(name=f"amC{block}", bufs=1),
                   tc.tile_pool(name=f"hC{block}", bufs=1),
                   tc.tile_pool(name=f"stC{block}", bufs=1),
                   tc.tile_pool(name=f"scrC{block}", bufs=2),
                   tc.tile_pool(name=f"psC{block}", bufs=6, space="PSUM"))
            return tuple(stack.enter_context(cm) for cm in cms)

        for rp in range(reps):
            # ---- block 0 ----
            with tc.tile_pool(name=f"qk0_{rp}", bufs=1) as qkp:
                q_sb = qkp.tile([D, B, N], FP8, tag="q")
                k_sb = qkp.tile([D, B, N], FP8, tag="k")
                q8 = qkp.tile([32, 2, B, N], FP8, tag="q8")
                k8 = qkp.tile([32, 2, B, N], FP8, tag="k8")
                vt_sb = qkp.tile([128, B, 16, D + 1], BF16, tag="vt")
                a_bf = qkp.tile([D, B, N], BF16, tag="abf")
                nc.vector.memset(vt_sb[:, :, :, D], 1.0)
                for b in range(B):
                    emit_kv_proj_block0(k_sb, k8, vt_sb, b)
                    emit_q_proj(src, q_sb, q8, b, f"q0_{rp}_{b}")
                with tc.tile_pool(name=f"pB0_{rp}", bufs=2) as pp, \
                     tc.tile_pool(name=f"dB0_{rp}", bufs=4) as dnp, \
                     tc.tile_pool(name=f"psS0_{rp}", bufs=2, space="PSUM") as psS, \
                     tc.tile_pool(name=f"psPV0_{rp}", bufs=2, space="PSUM") as psPV, \
                     tc.tile_pool(name=f"psT0_{rp}", bufs=2, space="PSUM") as psT:
                    for b in range(B):
                        emit_attention(0, b, (q_sb, q8), (k_sb, k8), vt_sb,
                                       a_bf, pp, dnp, psS, psPV, psT)
            # ---- block 1 ----  (q2 projection + batch-pipelined mlp0 fill
            # the A2A / AllReduce stalls)
            with tc.tile_pool(name=f"qk1_{rp}", bufs=1) as qkp:
                q2_sb = qkp.tile([D, B, N], FP8, tag="q2")
                q28 = qkp.tile([32, 2, B, N], FP8, tag="q28")
                for b in range(B):
                    emit_q_proj(tgt, q2_sb, q28, b, f"q1_{rp}_{b}")
                with bass.ExitStack() as st0:
                    pools0 = mlp_pools(f"0_{rp}", st0)
                    fr = {}
                    for b in range(B):
                        fr[b] = emit_mlp_front(0, b, x1c, x1b, pools0)
                    for b in range(B):
                        x_sb, _, h1_sb = fr[b]
                        emit_mlp_back(0, b, x_sb, h1_sb, src_out_c,
                                      True, pools0)
                k_sb = qkp.tile([D, B, N], FP8, tag="k")
                k28 = qkp.tile([32, 2, B, N], FP8, tag="k28")
                vt_sb = qkp.tile([128, B, 16, D + 1], BF16, tag="vt")
                a_bf = qkp.tile([D, B, N], BF16, tag="abf")
                nc.vector.memset(vt_sb[:, :, :, D], 1.0)
                with tc.tile_pool(name=f"pB1_{rp}", bufs=2) as pp, \
                     tc.tile_pool(name=f"dB1_{rp}", bufs=4) as dnp, \
                     tc.tile_pool(name=f"psS1_{rp}", bufs=2, space="PSUM") as psS, \
                     tc.tile_pool(name=f"psPV1_{rp}", bufs=2, space="PSUM") as psPV, \
                     tc.tile_pool(name=f"psT1_{rp}", bufs=2, space="PSUM") as psT:
                    for b in range(B):
                        emit_kv_assembly_block1(k_sb, k28, vt_sb, b)
                        emit_attention(1, b, (q2_sb, q28), (k_sb, k28), vt_sb,
                                       a_bf, pp, dnp, psS, psPV, psT)
            with bass.ExitStack() as st1:
                pools1 = mlp_pools(f"1_{rp}", st1)
                fr = {}
                for b in range(B):
                    fr[b] = emit_mlp_front(1, b, x2c, x2b, pools1)
                for b in range(B):
                    x_sb, _, h1_sb = fr[b]
                    emit_mlp_back(1, b, x_sb, h1_sb, tgt_out_c, False, pools1)

    nc.finalize()
    return nc


def _prep_inputs(src, tgt, Wq, bq, Wk, bk, Wv, bv, Wm, bm, W1, b1, W2, b2):
    """Host-side slicing/permutation/bf16-casting into the per-core in_maps."""
    BF = ml_dtypes.bfloat16
    src = np.ascontiguousarray(src, np.float32)
    tgt = np.ascontiguousarray(tgt, np.float32)
    src_bf = src.astype(BF)
    tgt_bf = tgt.astype(BF)
    perm = np.arange(F).reshape(D, H).T.reshape(F)  # f' = h*64+d -> f = d*8+h
    wm_perm = np.ascontiguousarray(
        np.asarray(Wm).reshape(F, D, H).transpose(2, 1, 0).reshape(F, F)).astype(BF)
    w1t = np.ascontiguousarray(np.asarray(W1).T).astype(BF)
    w2t = np.ascontiguousarray(np.asarray(W2).T).astype(BF)
    wk2t = np.ascontiguousarray(np.asarray(Wk)[perm, :].T).astype(BF)
    wv2t = np.ascontiguousarray(np.asarray(Wv)[perm, :].T).astype(BF)
    bk2 = np.ascontiguousarray(np.asarray(bk)[perm].reshape(F, 1), np.float32)
    bm_c = np.ascontiguousarray(np.asarray(bm).reshape(F, 1), np.float32)
    b2_c = np.ascontiguousarray(np.asarray(b2).reshape(F, 1), np.float32)
    in_maps = []
    for h in range(NCORES):
        f_list = np.arange(D) * H + h
        bqk_h = np.stack([np.asarray(bq)[f_list], np.asarray(bk)[f_list]],
                         axis=1).astype(np.float32)
        sl = slice(h * NL, (h + 1) * NL)
        in_maps.append({
            "src": src_bf,
            "tgt": tgt_bf,
            "x1c": np.ascontiguousarray(src[:, :, sl]),
            "x2c": np.ascontiguousarray(tgt[:, :, sl]),
            "x1b": np.ascontiguousarray(src_bf[:, :, sl]),
            "x2b": np.ascontiguousarray(tgt_bf[:, :, sl]),
            "wq": np.ascontiguousarray(np.asarray(Wq)[f_list, :].T).astype(BF),
            "wk": np.ascontiguousarray(np.asarray(Wk)[f_list, :].T).astype(BF),
            "wv": np.ascontiguousarray(np.asarray(Wv)[f_list, :].T).astype(BF),
            "bqk": np.ascontiguousarray(bqk_h),
            "bv": np.ascontiguousarray(
                np.asarray(bv)[f_list].reshape(D, 1), np.float32),
            "wmT": wm_perm,
            "bm": bm_c,
            "w1T": w1t,
            "w2T": w2t,
            "b2": b2_c,
            "wk2T": wk2t,
            "wv2T": wv2t,
            "bk2": bk2,
        })
    return in_maps


def kernel(**inputs):
    if "nc" not in _CACHE:
        _CACHE["nc"] = build_nc()
    nc = _CACHE["nc"]
    in_maps = _prep_inputs(**inputs)
    res = run_bass_kernel_spmd(nc, in_maps, list(range(NCORES)))
    src_out = np.concatenate(
        [res.results[c]["src_out_c"] for c in range(NCORES)], axis=2)
    tgt_out = np.concatenate(
        [res.results[c]["tgt_out_c"] for c in range(NCORES)], axis=2)
    return (src_out, tgt_out)
